# revision 1
# baseline (speedup 1.0000x reference)
"""ExpertScatter TRN2 kernel.

reference semantics:
    X = einsum('bekj,eji->beki', Y, W)          # per-head projection
    out[b] = zeros([T, I]); out[b, Ind[b,e,k]] += X[b,e,k]

Strategy (data-parallel over batch, 1 batch per NeuronCore):
  Phase A: per head e, matmul X_chunk[128 rows, 1024] = Yt_chunk.T @ W[e]
           (fp16 operands by default; float32r available = full PE rate
           with fp32 data), write X to an internal HBM staging buffer in
           natural row order (fp16 halves the round-trip traffic).
  Host precomputes a global sort of the 16384 rows of each batch by target
  slot, padded to a fixed PT rows per 128-slot output tile.
  Phase B: per output tile (128 slots), dma_gather the contributing rows
           (PT of them) into SBUF, build one-hot selection matrices on DVE
           (is_equal against a column-iota constant), and accumulate
           out_tile = sum_g onehot_g.T @ Xrows_g in PSUM. One DMA per tile
           writes the finished [128, 1024] block of the output.

All shapes/counts are identical across cores (SPMD); per-core data
differences live entirely in the input tensors (Yt, gather indices,
relative-column tables).
"""

import os

import numpy as np

import concourse.bacc as bacc
import concourse.mybir as mybir
import concourse.tile as tile
from concourse.bass_utils import run_bass_kernel_spmd

# Problem constants (hardcoded per harness contract).
B = 8
HEADS = 16
K = 1024
HEAD_DIM = 128
OUT_DIM = 1024
T_SLOTS = 4096

R = HEADS * K            # rows per batch = 16384
NT = T_SLOTS // 128      # output tiles per batch = 32
PT = 640                 # gather buffer rows per output tile (5 groups)
NG = PT // 128           # row groups (matmuls) per output tile = 5
NCORES = 8

F32 = mybir.dt.float32
F32R = mybir.dt.float32r
BF16 = mybir.dt.bfloat16
FP16 = mybir.dt.float16
I16 = mybir.dt.int16

# Projection matmul dtype: "f32r" (full-rate fp32), "f32" (4x slower),
# or "fp16" (halves Y/W traffic, ~2x err).
MM_DTYPE = os.environ.get("ES_MM_DTYPE", "fp16")
MM_F32R = MM_DTYPE == "f32r"
# X staging / scatter dtype: "fp16", "bf16", "f32r", or "f32".
X_DTYPE = os.environ.get("ES_X_DTYPE", "fp16")
# Debug: which phases to emit ("AB", "A", "B").
PHASES = os.environ.get("ES_PHASES", "AB")
# Scheduling knobs.
GBUFS = int(os.environ.get("ES_GBUFS", "4"))
XBUFS = int(os.environ.get("ES_XBUFS", "6"))
WSPLIT = os.environ.get("ES_WSPLIT", "1") == "1"
BARRIER = os.environ.get("ES_BARRIER", "0") == "1"
# Write the output in fp16 (host casts back to f32): halves out traffic.
OUT_FP16 = os.environ.get("ES_OUT_FP16", "1") == "1"
# Alternate PSUM->SBUF copies between DVE and ACT.
ALT_COPY = os.environ.get("ES_ALT_COPY", "1") == "1"
# Trailing -1 index padding (skipped by Q7 -> less gather traffic). Found
# unreliable on HW at full scale (intermittent NRT faults) -> default off.
EXACT_CNT = os.environ.get("ES_EXACT_CNT", "0") == "1"

_cache = {}


def _build_program(mdt, sdt, gnum):
    """mdt: projection matmul dtype; sdt: X staging + scatter dtype;
    gnum: gathered positions per tile (<= PT; rest is never read thanks to
    the one-hot sentinel, but must hold finite values)."""
    nc = bacc.Bacc("TRN2", target_bir_lowering=False, debug=False,
                   num_devices=NCORES)

    yt = nc.dram_tensor("yt", [HEAD_DIM, R], mdt, kind="ExternalInput").ap()
    w = nc.dram_tensor("w", [HEAD_DIM, HEADS * OUT_DIM], mdt,
                       kind="ExternalInput").ap()
    gidx = nc.dram_tensor("gidx", [128, NT * (PT // 16)], I16,
                          kind="ExternalInput").ap()
    relc = nc.dram_tensor("relc", [128, NT * NG], F32,
                          kind="ExternalInput").ap()
    cols = nc.dram_tensor("cols", [128, 128], F32, kind="ExternalInput").ap()
    odt = FP16 if OUT_FP16 else F32
    out = nc.dram_tensor("out", [T_SLOTS, OUT_DIM], odt,
                         kind="ExternalOutput").ap()
    xnat = nc.dram_tensor("xnat", [R, OUT_DIM], sdt).ap()

    with tile.TileContext(nc) as tc:
        with (
            tc.tile_pool(name="const", bufs=1) as cpool,
            tc.tile_pool(name="yhead",
                         bufs=int(os.environ.get("ES_YBUFS", "2"))) as ypool,
            tc.tile_pool(name="xchunk", bufs=XBUFS) as xpool,
            tc.tile_pool(name="gather", bufs=GBUFS) as gpool,
            tc.tile_pool(name="onehot",
                         bufs=int(os.environ.get("ES_OHBUFS", "4"))) as ohpool,
            tc.tile_pool(name="otile",
                         bufs=int(os.environ.get("ES_OBUFS", "4"))) as opool,
        ):
            w_sb = cpool.tile([128, HEADS * OUT_DIM], mdt, tag="w")
            if WSPLIT:
                for e in range(HEADS):
                    nc.sync.dma_start(
                        out=w_sb[:, e * OUT_DIM:(e + 1) * OUT_DIM],
                        in_=w[:, e * OUT_DIM:(e + 1) * OUT_DIM])
            else:
                nc.sync.dma_start(out=w_sb[:], in_=w[:])
            gidx_sb = cpool.tile([128, NT * (PT // 16)], I16, tag="gidx")
            nc.sync.dma_start(out=gidx_sb[:], in_=gidx[:])
            relc_sb = cpool.tile([128, NT * NG], F32, tag="relc")
            nc.sync.dma_start(out=relc_sb[:], in_=relc[:])
            cols_sb = cpool.tile([128, 128], F32, tag="cols")
            nc.sync.dma_start(out=cols_sb[:], in_=cols[:])

            # ---- Phase A: projection, X written to HBM in natural order --
            pa_ctx = tc.tile_pool(name="psumA",
                                  bufs=int(os.environ.get("ES_PABUFS", "2")),
                                  space="PSUM")
            pspool = pa_ctx.__enter__()
            for e in range(HEADS if "A" in PHASES else 0):
                yt_e = ypool.tile([128, K], mdt, tag="yt")
                nc.sync.dma_start(out=yt_e[:], in_=yt[:, e * K:(e + 1) * K])
                for rc in range(K // 128):
                    px = pspool.tile([128, OUT_DIM], F32, tag="pa")
                    lhsT = yt_e[:, rc * 128:(rc + 1) * 128]
                    for h in range(2):
                        nc.tensor.matmul(
                            out=px[:, h * 512:(h + 1) * 512],
                            lhsT=lhsT,
                            rhs=w_sb[:, e * OUT_DIM + h * 512:
                                     e * OUT_DIM + (h + 1) * 512],
                            start=True, stop=True,
                        )
                    xc = xpool.tile([128, OUT_DIM], sdt, tag="xc")
                    if ALT_COPY and rc % 2 == 1:
                        nc.scalar.copy(out=xc[:], in_=px[:])
                    else:
                        nc.vector.tensor_copy(out=xc[:], in_=px[:])
                    row0 = (e * (K // 128) + rc) * 128
                    xeng = (nc.scalar if os.environ.get("ES_DMAALT", "0") == "1"
                            and rc % 2 == 0 else nc.sync)
                    xeng.dma_start(out=xnat[row0:row0 + 128, :], in_=xc[:])

            pa_ctx.__exit__(None, None, None)

            # Fence: every gather below reads rows written above.
            if BARRIER and "A" in PHASES and "B" in PHASES:
                tc.strict_bb_all_engine_barrier()
            pb_ctx = tc.tile_pool(name="psumB",
                                  bufs=int(os.environ.get("ES_PBBUFS", "2")),
                                  space="PSUM")
            pspool = pb_ctx.__enter__()

            # ---- Phase B: gather sorted rows per tile, one-hot matmul ----
            splitg = os.environ.get("ES_SPLITG", "1") == "1"
            for t in range(NT if "B" in PHASES else 0):
                g = gpool.tile([128, NG, OUT_DIM], sdt, tag="g")
                if EXACT_CNT and t < GBUFS:
                    # With -1 skip-padding, unwritten positions vary per
                    # tile; scrub whole fresh slots once so unread regions
                    # hold finite values (one-hot sentinel zeroes them).
                    nc.gpsimd.memset(g[:], 0.0)
                elif gnum < PT and t < GBUFS:
                    # Positions gnum..PT are never gathered; scrub the
                    # fresh SBUF slots once so the unread region holds
                    # finite values (one-hot sentinel zeroes them out).
                    lastp = (gnum // 128) * 128
                    nc.gpsimd.memset(g[gnum - lastp:, NG - 1, :], 0.0)
                gq = (t % 2) if os.environ.get("ES_GQALT", "0") == "1" else 0
                if splitg:
                    cut = int(os.environ.get("ES_GCUT", "384"))
                    nc.gpsimd.dma_gather(
                        out_ap=g[:, 0:cut // 128, :],
                        in_ap=xnat[:],
                        idxs_ap=gidx_sb[:, t * (PT // 16):
                                        t * (PT // 16) + cut // 16],
                        num_idxs=cut, num_idxs_reg=cut, elem_size=OUT_DIM,
                        queue_num=gq,
                    )
                    nc.gpsimd.dma_gather(
                        out_ap=g[:, cut // 128:NG, :],
                        in_ap=xnat[:],
                        idxs_ap=gidx_sb[:, t * (PT // 16) + cut // 16:
                                        t * (PT // 16) + gnum // 16],
                        num_idxs=gnum - cut, num_idxs_reg=gnum - cut,
                        elem_size=OUT_DIM, queue_num=gq,
                    )
                else:
                    nc.gpsimd.dma_gather(
                        out_ap=g[:],
                        in_ap=xnat[:],
                        idxs_ap=gidx_sb[:, t * (PT // 16):
                                        t * (PT // 16) + gnum // 16],
                        num_idxs=gnum,
                        num_idxs_reg=gnum,
                        elem_size=OUT_DIM,
                    )
                pt = pspool.tile([128, OUT_DIM], F32, tag="pb")
                for gi in range(NG):
                    oh = ohpool.tile([128, 128], sdt, tag="oh")
                    c = t * NG + gi
                    nc.vector.tensor_tensor(
                        out=oh[:],
                        in0=relc_sb[:, c:c + 1].to_broadcast([128, 128]),
                        in1=cols_sb[:],
                        op=mybir.AluOpType.is_equal,
                    )
                    for h in range(2):
                        nc.tensor.matmul(
                            out=pt[:, h * 512:(h + 1) * 512],
                            lhsT=oh[:],
                            rhs=g[:, gi, h * 512:(h + 1) * 512],
                            start=(gi == 0), stop=(gi == NG - 1),
                        )
                ot = opool.tile([128, OUT_DIM], odt, tag="ot")
                if ALT_COPY and t % 2 == 1:
                    nc.scalar.copy(out=ot[:], in_=pt[:])
                else:
                    nc.vector.tensor_copy(out=ot[:], in_=pt[:])
                nc.sync.dma_start(out=out[t * 128:(t + 1) * 128, :], in_=ot[:])
            pb_ctx.__exit__(None, None, None)

    nc.compile()
    return nc


def _get_program(gnum=576):
    mdt = {"f32r": F32R, "f32": F32, "fp16": FP16, "bf16": BF16}[MM_DTYPE]
    sdt = {"f32r": F32R if MM_F32R else F32, "f32": F32,
           "bf16": BF16, "fp16": FP16}[X_DTYPE]
    key = (MM_DTYPE, X_DTYPE, PHASES, GBUFS, XBUFS, WSPLIT, BARRIER,
           ALT_COPY, EXACT_CNT, OUT_FP16, gnum,
           os.environ.get("ES_SPLITG", "1"),
           os.environ.get("ES_OBUFS", "4"), os.environ.get("ES_YBUFS", "2"),
           os.environ.get("ES_PABUFS", "2"), os.environ.get("ES_PBBUFS", "2"))
    if key not in _cache:
        _cache[key] = _build_program(mdt, sdt, gnum)
    return _cache[key]


def _prep_core_inputs(Yb, Indb):
    """Host-side prep for one batch: transpose Y, sort rows by slot,
    build padded gather-index and relative-column tables."""
    yt = np.ascontiguousarray(
        Yb.transpose(2, 0, 1).reshape(HEAD_DIM, R)).astype(np.float32)
    ind = Indb.reshape(R).astype(np.int64)
    order = np.argsort(ind, kind="stable")
    sind = ind[order]
    counts = np.bincount(sind // 128, minlength=NT)
    assert counts.max() <= PT, f"tile overflow: {counts.max()} > {PT}"
    _prep_core_inputs.max_count = max(
        getattr(_prep_core_inputs, "max_count", 0), int(counts.max()))
    pad = -1 if EXACT_CNT else 0
    gidx = np.full((NT, PT), pad, dtype=np.int16)
    relc = np.full((NT, PT), -1000.0, dtype=np.float32)
    pos = 0
    for t in range(NT):
        c = counts[t]
        gidx[t, :c] = order[pos:pos + c]
        relc[t, :c] = (sind[pos:pos + c] - t * 128).astype(np.float32)
        pos += c
    # dma_gather index layout: position p -> (partition p%16, col p//16),
    # and the 16-partition block replicated across all 8 Q7 core groups.
    blk = np.concatenate(
        [gidx[t].reshape(PT // 16, 16).T for t in range(NT)], axis=1)
    gidx_sb = np.ascontiguousarray(np.tile(blk, (8, 1)), dtype=np.int16)
    # one-hot layout: position p -> (partition p%128, group p//128)
    relc_sb = np.concatenate(
        [relc[t].reshape(NG, 128).T for t in range(NT)], axis=1)
    relc_sb = np.ascontiguousarray(relc_sb, dtype=np.float32)
    return yt, gidx_sb, relc_sb


def kernel(Y, Ind, T, W):
    Y = np.asarray(Y, dtype=np.float32)
    Ind = np.asarray(Ind)
    W = np.asarray(W, dtype=np.float32)
    assert int(T) == T_SLOTS and Y.shape == (B, HEADS, K, HEAD_DIM)

    if MM_DTYPE == "fp16":
        np_mdt = np.float16
    elif MM_DTYPE == "bf16":
        import ml_dtypes
        np_mdt = ml_dtypes.bfloat16
    else:
        np_mdt = np.float32
    w_in = np.ascontiguousarray(
        W.transpose(1, 0, 2).reshape(HEAD_DIM, HEADS * OUT_DIM)
    ).astype(np_mdt)
    cols_in = np.broadcast_to(
        np.arange(128, dtype=np.float32)[None, :], (128, 128)).copy()

    _prep_core_inputs.max_count = 0
    in_maps = []
    for b in range(B):
        yt, gidx_sb, relc_sb = _prep_core_inputs(Y[b], Ind[b])
        in_maps.append({
            "yt": yt.astype(np_mdt), "w": w_in, "gidx": gidx_sb,
            "relc": relc_sb, "cols": cols_in,
        })
    gnum = 576 if _prep_core_inputs.max_count <= 576 else PT
    nc = _get_program(gnum)

    # The first execution of a freshly compiled NEFF occasionally wedges a
    # core (NRT_EXEC_UNIT_UNRECOVERABLE); a retry on a fresh execute has
    # been observed to recover.
    last_exc = None
    for attempt in range(3):
        try:
            res = run_bass_kernel_spmd(
                nc, in_maps, core_ids=list(range(NCORES)),
                trace=os.environ.get("ES_TRACE", "0") == "1",
            )
            break
        except Exception as exc:  # noqa: BLE001 - device flake, retry
            last_exc = exc
            import time as _time
            _time.sleep(2.0)
    else:
        raise last_exc
    kernel.last_results = res
    out = np.stack([res.results[b]["out"] for b in range(B)], axis=0)
    return out.astype(np.float32)



# revision 10
# speedup vs baseline: 1.7579x; 1.7579x over previous
"""ExpertScatter TRN2 kernel — DMA scatter-add design.

reference semantics:
    X = einsum('bekj,eji->beki', Y, W)          # per-head projection
    out[b] = zeros([T, I]); out[b, Ind[b,e,k]] += X[b,e,k]

Strategy (data-parallel over batch, 1 batch per NeuronCore):
  The projection is linear, so Y rows of one head that target the same
  slot are combined on the HOST (summed before the matmul). After that,
  every head's <=1024 virtual rows have DISTINCT target slots.

  Phase A: per head e, matmul X_chunk[128 rows, 1024] = Yt_chunk.T @ W[e]
           (fp16 operands = full PE rate), copy PSUM -> SBUF fp16 split
           into two column halves.
  Scatter: per head, dma_scatter_add (SWDGE CCE add) writes
           out[slot] += X_row straight from SBUF to the output in HBM.
           Within one head all slots are distinct -> no same-address race
           inside an op. Across heads, ops on the same column half are
           chained with DMA-completion semaphores (op for head e waits
           until head e-1's last byte landed). Two independent chains
           (column halves 0:512 / 512:1024, disjoint HBM ranges, separate
           output tensors + SWDGE queues) interleave so the DMA engines
           never idle during a chain barrier.

  The PJRT execution path donates zero-initialized buffers for
  ExternalOutputs, so out starts at exactly 0.0 and needs no zero-fill.

All shapes/counts are identical across cores (SPMD); per-core data
differences live entirely in the input tensors.
"""

import os

import numpy as np

import concourse.bacc as bacc
import concourse.mybir as mybir
import concourse.tile as tile
from concourse.bass_utils import run_bass_kernel_spmd

# Problem constants (hardcoded per harness contract).
B = 8
HEADS = 16
K = 1024
HEAD_DIM = 128
OUT_DIM = 1024
T_SLOTS = 4096

NCORES = 8
HALF = OUT_DIM // 2          # column half width = 512
SUB = 2                      # scatter sub-ops per head per chain
SUBN = K // SUB              # idxs per sub-op = 512

F32 = mybir.dt.float32
FP16 = mybir.dt.float16
I16 = mybir.dt.int16

_cache = {}


USE_SEMS = os.environ.get("ES_SEMS", "0") == "1"
NQUEUES = int(os.environ.get("ES_QUEUES", "1"))


def _build_program():
    nc = bacc.Bacc("TRN2", target_bir_lowering=False, debug=False,
                   num_devices=NCORES, num_swdge_queues=NQUEUES)

    yt = nc.dram_tensor("yt", [HEAD_DIM, HEADS * K], FP16,
                        kind="ExternalInput").ap()
    w = nc.dram_tensor("w", [HEAD_DIM, HEADS * OUT_DIM], FP16,
                       kind="ExternalInput").ap()
    gidx = nc.dram_tensor("gidx", [128, HEADS * (K // 16)], I16,
                          kind="ExternalInput").ap()
    out_lo = nc.dram_tensor("out_lo", [T_SLOTS, HALF], FP16,
                            kind="ExternalOutput").ap()
    out_hi = nc.dram_tensor("out_hi", [T_SLOTS, HALF], FP16,
                            kind="ExternalOutput").ap()

    sem_lo = nc.alloc_semaphore("chain_lo")
    sem_hi = nc.alloc_semaphore("chain_hi")

    ybufs = int(os.environ.get("ES_YBUFS", "3"))
    xbufs = int(os.environ.get("ES_XBUFS", "2"))
    pabufs = int(os.environ.get("ES_PABUFS", "3"))

    with tile.TileContext(nc) as tc:
        with (
            tc.tile_pool(name="const", bufs=1) as cpool,
            tc.tile_pool(name="yhead", bufs=ybufs) as ypool,
            tc.tile_pool(name="xstage", bufs=xbufs) as xpool,
            tc.tile_pool(name="psumA", bufs=pabufs, space="PSUM") as pspool,
        ):
            w_sb = cpool.tile([128, HEADS * OUT_DIM], FP16, tag="w")
            for e in range(HEADS):
                nc.sync.dma_start(
                    out=w_sb[:, e * OUT_DIM:(e + 1) * OUT_DIM],
                    in_=w[:, e * OUT_DIM:(e + 1) * OUT_DIM])
            gidx_sb = cpool.tile([128, HEADS * (K // 16)], I16, tag="gidx")
            nc.sync.dma_start(out=gidx_sb[:], in_=gidx[:])

            for e in range(HEADS):
                yt_e = ypool.tile([128, K], FP16, tag="yt")
                nc.sync.dma_start(out=yt_e[:], in_=yt[:, e * K:(e + 1) * K])
                xa = xpool.tile([128, K // 128, HALF], FP16, tag="xa")
                xb = xpool.tile([128, K // 128, HALF], FP16, tag="xb")
                for rc in range(K // 128):
                    px = pspool.tile([128, OUT_DIM], F32, tag="pa")
                    lhsT = yt_e[:, rc * 128:(rc + 1) * 128]
                    for h in range(2):
                        nc.tensor.matmul(
                            out=px[:, h * HALF:(h + 1) * HALF],
                            lhsT=lhsT,
                            rhs=w_sb[:, e * OUT_DIM + h * HALF:
                                     e * OUT_DIM + (h + 1) * HALF],
                            start=True, stop=True,
                        )
                    nc.vector.tensor_copy(out=xa[:, rc, :], in_=px[:, :HALF])
                    nc.scalar.copy(out=xb[:, rc, :], in_=px[:, HALF:])

                # chain barriers: head e's scatters wait until head e-1's
                # scatters on the same output half fully landed (16 sem incs
                # per sub-op, SUB sub-ops per head per chain).
                for chain, (xt, out_t, sem) in enumerate(
                        ((xa, out_lo, sem_lo), (xb, out_hi, sem_hi))):
                    for s in range(SUB):
                        col0 = e * (K // 16) + s * (SUBN // 16)
                        inst = nc.gpsimd.dma_scatter_add(
                            out_ap=out_t[:],
                            in_ap=xt[:, s * (SUBN // 128):
                                     (s + 1) * (SUBN // 128), :],
                            idxs_ap=gidx_sb[:, col0:col0 + SUBN // 16],
                            num_idxs=SUBN,
                            num_idxs_reg=SUBN,
                            elem_size=HALF,
                            queue_num=chain % NQUEUES,
                        )
                        if USE_SEMS:
                            if e > 0:
                                inst._wait_ge(sem, 16 * SUB * e)
                            inst.then_inc(sem, 16)

    nc.compile()
    return nc


def _get_program():
    key = ("v2", USE_SEMS, NQUEUES)
    if key not in _cache:
        _cache[key] = _build_program()
    return _cache[key]


def _prep_core_inputs(Yb, Indb):
    """Host-side prep for one batch: per head, combine duplicate-slot rows
    (projection is linear), transpose to [HEAD_DIM, K] fp16, build the
    wrapped int16 index table."""
    yt = np.zeros((HEAD_DIM, HEADS * K), dtype=np.float16)
    idx = np.zeros((HEADS, K), dtype=np.int16)
    # Preferred pad target: a slot no head of this core ever touches, so
    # pad traffic can never race with real contributions even across ops.
    unused = np.setdiff1d(np.arange(T_SLOTS), np.asarray(Indb).reshape(-1))
    glob_trash = int(unused[0]) if len(unused) else -1
    for e in range(HEADS):
        ind = np.asarray(Indb[e], dtype=np.int64)
        u, inv = np.unique(ind, return_inverse=True)
        summed = np.zeros((len(u), HEAD_DIM), dtype=np.float32)
        np.add.at(summed, inv, np.asarray(Yb[e], dtype=np.float32))
        yt[:, e * K:e * K + len(u)] = summed.T.astype(np.float16)
        idx[e, :len(u)] = u.astype(np.int16)
        # Tail pad rows carry zero values, but a pad's CCE read-add-write
        # still races with a REAL row of the same op targeting the same
        # slot (the pad can write back a stale value). Point pads at a
        # slot this head never touches (adds +0.0 there, harmless).
        trash = glob_trash if glob_trash >= 0 else int(
            np.setdiff1d(np.arange(T_SLOTS), u)[0])
        idx[e, len(u):] = np.int16(trash)
    # dma index layout per head: position p -> (partition p%16, col p//16),
    # 16-partition block replicated across all 8 Q7 core groups.
    blk = np.concatenate(
        [idx[e].reshape(K // 16, 16).T for e in range(HEADS)], axis=1)
    gidx_sb = np.ascontiguousarray(np.tile(blk, (8, 1)), dtype=np.int16)
    return yt, gidx_sb


def kernel(Y, Ind, T, W):
    Y = np.asarray(Y, dtype=np.float32)
    Ind = np.asarray(Ind)
    W = np.asarray(W, dtype=np.float32)
    assert int(T) == T_SLOTS and Y.shape == (B, HEADS, K, HEAD_DIM)

    w_in = np.ascontiguousarray(
        W.transpose(1, 0, 2).reshape(HEAD_DIM, HEADS * OUT_DIM)
    ).astype(np.float16)

    in_maps = []
    for b in range(B):
        yt, gidx_sb = _prep_core_inputs(Y[b], Ind[b])
        in_maps.append({"yt": yt, "w": w_in, "gidx": gidx_sb})
    nc = _get_program()

    # The first execution of a freshly compiled NEFF occasionally wedges a
    # core; a retry on a fresh execute has been observed to recover.
    last_exc = None
    for attempt in range(3):
        try:
            res = run_bass_kernel_spmd(
                nc, in_maps, core_ids=list(range(NCORES)),
                trace=os.environ.get("ES_TRACE", "0") == "1",
            )
            break
        except Exception as exc:  # noqa: BLE001 - device flake, retry
            last_exc = exc
            import time as _time
            _time.sleep(2.0)
    else:
        raise last_exc
    kernel.last_results = res
    out = np.empty((B, T_SLOTS, OUT_DIM), dtype=np.float32)
    for b in range(B):
        out[b, :, :HALF] = res.results[b]["out_lo"].astype(np.float32)
        out[b, :, HALF:] = res.results[b]["out_hi"].astype(np.float32)
    return out


if __name__ == "__main__":
    # quick self-check against a numpy reference
    rng = np.random.default_rng(0)
    Y = rng.standard_normal((B, HEADS, K, HEAD_DIM)).astype(np.float32)
    Ind = rng.integers(0, T_SLOTS, (B, HEADS, K)).astype(np.int32)
    bound = 1.0 / np.sqrt(OUT_DIM * HEADS)
    W = rng.uniform(-bound, bound, (HEADS, HEAD_DIM, OUT_DIM)).astype(np.float32)
    got = kernel(Y, Ind, T_SLOTS, W)
    X = np.einsum("bekj,eji->beki", Y.astype(np.float64), W.astype(np.float64))
    exp = np.zeros((B, T_SLOTS, OUT_DIM))
    for b in range(B):
        np.add.at(exp[b], Ind[b].reshape(-1), X[b].reshape(-1, OUT_DIM))
    err = np.linalg.norm(got - exp) / np.linalg.norm(exp)
    print(f"rel err {err:.3e}")


# revision 24
# speedup vs baseline: 2.3508x; 1.3373x over previous
"""ExpertScatter TRN2 kernel — DMA scatter-add design.

reference semantics:
    X = einsum('bekj,eji->beki', Y, W)          # per-head projection
    out[b] = zeros([T, I]); out[b, Ind[b,e,k]] += X[b,e,k]

Strategy (data-parallel over batch, 1 batch per NeuronCore):
  The projection is linear, so Y rows of one head that target the same
  slot are combined on the HOST (summed before the matmul). After that,
  every head's <=1024 virtual rows have DISTINCT target slots.

  Phase A: per head e, matmul X_chunk[128 rows, 1024] = Yt_chunk.T @ W[e]
           (fp16 operands = full PE rate), copy PSUM -> SBUF fp16 split
           into two column halves.
  Scatter: per head, dma_scatter_add (SWDGE CCE add) writes
           out[slot] += X_row straight from SBUF to the output in HBM.
           Within one head all slots are distinct -> no same-address race
           inside an op. Across heads, ops on the same column half are
           chained with DMA-completion semaphores (op for head e waits
           until head e-1's last byte landed). Two independent chains
           (column halves 0:512 / 512:1024, disjoint HBM ranges, separate
           output tensors + SWDGE queues) interleave so the DMA engines
           never idle during a chain barrier.

  The PJRT execution path donates zero-initialized buffers for
  ExternalOutputs, so out starts at exactly 0.0 and needs no zero-fill.

All shapes/counts are identical across cores (SPMD); per-core data
differences live entirely in the input tensors.
"""

import os

import numpy as np

import concourse.bacc as bacc
import concourse.mybir as mybir
import concourse.tile as tile
from concourse.bass_utils import run_bass_kernel_spmd

# Problem constants (hardcoded per harness contract).
B = 8
HEADS = 16
K = 1024
HEAD_DIM = 128
OUT_DIM = 1024
T_SLOTS = 4096

NCORES = 8
# Output striping:
#  - NCHAINS column stripes (separate ExternalOutput tensors) — disjoint
#    HBM column ranges.
#  - NPARITY accumulation buffers per stripe; head e scatters into buffer
#    e % NPARITY and the host sums the buffers afterwards. Consecutive
#    heads therefore write DIFFERENT tensors (no ordering needed), while
#    same-buffer heads are 2 apart — Tile's WAW completion chain orders
#    them, but that predecessor finished long ago, so the barrier latency
#    is fully hidden and the DMA engines run back-to-back.
NCHAINS = int(os.environ.get("ES_NCHAINS", "1"))
NPARITY = int(os.environ.get("ES_NPARITY", "2"))
CW = OUT_DIM // NCHAINS      # chain column width
SUB = int(os.environ.get("ES_SUB", "1"))  # scatter sub-ops per head per chain
SUBN = K // SUB

F32 = mybir.dt.float32
FP16 = mybir.dt.float16
I16 = mybir.dt.int16

_cache = {}


USE_SEMS = os.environ.get("ES_SEMS", "0") == "1"
NQUEUES = int(os.environ.get("ES_QUEUES", "1"))


def _build_program():
    nc = bacc.Bacc("TRN2", target_bir_lowering=False, debug=False,
                   num_devices=NCORES, num_swdge_queues=NQUEUES,
                   dynamic_dma_scratch_size=int(
                       os.environ.get("ES_SCRATCH", "65536")))

    yt = nc.dram_tensor("yt", [HEAD_DIM, HEADS * K], FP16,
                        kind="ExternalInput").ap()
    w = nc.dram_tensor("w", [HEAD_DIM, HEADS * OUT_DIM], FP16,
                       kind="ExternalInput").ap()
    gidx = nc.dram_tensor("gidx", [128, HEADS * (K // 16)], I16,
                          kind="ExternalInput").ap()
    outs = [
        [nc.dram_tensor(f"out{c}_{p}", [T_SLOTS, CW], FP16,
                        kind="ExternalOutput").ap()
         for p in range(NPARITY)]
        for c in range(NCHAINS)
    ]

    ybufs = int(os.environ.get("ES_YBUFS", "3"))
    xbufs = int(os.environ.get("ES_XBUFS", "3"))
    pabufs = int(os.environ.get("ES_PABUFS", "3"))

    with tile.TileContext(nc) as tc:
        with (
            tc.tile_pool(name="const", bufs=1) as cpool,
            tc.tile_pool(name="yhead", bufs=ybufs) as ypool,
            tc.tile_pool(name="xstage", bufs=xbufs) as xpool,
            tc.tile_pool(name="psumA", bufs=pabufs, space="PSUM") as pspool,
        ):
            w_sb = cpool.tile([128, HEADS * OUT_DIM], FP16, tag="w")
            gidx_sb = cpool.tile([128, HEADS * (K // 16)], I16, tag="gidx")

            def load_w(e):
                nc.sync.dma_start(
                    out=w_sb[:, e * OUT_DIM:(e + 1) * OUT_DIM],
                    in_=w[:, e * OUT_DIM:(e + 1) * OUT_DIM])

            def load_y(e):
                yt_e = ypool.tile([128, K], FP16, tag="yt", name=f"yt_{e}")
                nc.sync.dma_start(out=yt_e[:], in_=yt[:, e * K:(e + 1) * K])
                return yt_e

            # Head 0's operands first so the PE starts ~3us in; the
            # remaining W/Y parts prefetch while earlier heads compute.
            nc.sync.dma_start(out=gidx_sb[:], in_=gidx[:])
            yt_next = load_y(0)
            load_w(0)

            for e in range(HEADS):
                yt_e = yt_next
                if e + 1 < HEADS:
                    load_w(e + 1)
                    yt_next = load_y(e + 1)
                xts = []
                for c in range(NCHAINS):
                    xc = xpool.tile([128, K // 128, CW], FP16, tag=f"x{c}",
                                    name=f"x{c}_{e}")
                    xts.append(xc)
                for rc in range(K // 128):
                    px = pspool.tile([128, OUT_DIM], F32, tag="pa")
                    lhsT = yt_e[:, rc * 128:(rc + 1) * 128]
                    for h in range(2):
                        nc.tensor.matmul(
                            out=px[:, h * 512:(h + 1) * 512],
                            lhsT=lhsT,
                            rhs=w_sb[:, e * OUT_DIM + h * 512:
                                     e * OUT_DIM + (h + 1) * 512],
                            start=True, stop=True,
                        )
                    for c in range(NCHAINS):
                        eng = (nc.vector.tensor_copy if (rc * NCHAINS + c) % 2
                               else nc.scalar.copy)
                        eng(out=xts[c][:, rc, :],
                            in_=px[:, c * CW:(c + 1) * CW])

                # Tile inserts WAW sync deps between scatters writing the
                # same output tensor (wait = predecessor's DMA completion);
                # that is the cross-head ordering the CCE read-add-write
                # needs. Within one op all slots are distinct (host combine).
                for c in range(NCHAINS):
                    for s in range(SUB):
                        col0 = e * (K // 16) + s * (SUBN // 16)
                        nc.gpsimd.dma_scatter_add(
                            out_ap=outs[c][e % NPARITY][:],
                            in_ap=xts[c][:, s * (SUBN // 128):
                                         (s + 1) * (SUBN // 128), :],
                            idxs_ap=gidx_sb[:, col0:col0 + SUBN // 16],
                            num_idxs=SUBN,
                            num_idxs_reg=SUBN,
                            elem_size=CW,
                            queue_num=c % NQUEUES,
                        )

    nc.compile()
    return nc


def _get_program():
    key = ("v3", USE_SEMS, NQUEUES, NCHAINS, NPARITY, SUB)
    if key not in _cache:
        _cache[key] = _build_program()
    return _cache[key]


def _prep_core_inputs(Yb, Indb):
    """Host-side prep for one batch: per head, combine duplicate-slot rows
    (projection is linear), transpose to [HEAD_DIM, K] fp16, build the
    wrapped int16 index table."""
    yt = np.zeros((HEAD_DIM, HEADS * K), dtype=np.float16)
    idx = np.zeros((HEADS, K), dtype=np.int16)
    # Preferred pad target: a slot no head of this core ever touches, so
    # pad traffic can never race with real contributions even across ops.
    unused = np.setdiff1d(np.arange(T_SLOTS), np.asarray(Indb).reshape(-1))
    glob_trash = int(unused[0]) if len(unused) else -1
    for e in range(HEADS):
        ind = np.asarray(Indb[e], dtype=np.int64)
        u, inv = np.unique(ind, return_inverse=True)
        summed = np.zeros((len(u), HEAD_DIM), dtype=np.float32)
        np.add.at(summed, inv, np.asarray(Yb[e], dtype=np.float32))
        yt[:, e * K:e * K + len(u)] = summed.T.astype(np.float16)
        idx[e, :len(u)] = u.astype(np.int16)
        # Tail pad rows carry zero values, but a pad's CCE read-add-write
        # still races with a REAL row of the same op targeting the same
        # slot (the pad can write back a stale value). Point pads at a
        # slot this head never touches (adds +0.0 there, harmless).
        trash = glob_trash if glob_trash >= 0 else int(
            np.setdiff1d(np.arange(T_SLOTS), u)[0])
        idx[e, len(u):] = np.int16(trash)
    # dma index layout per head: position p -> (partition p%16, col p//16),
    # 16-partition block replicated across all 8 Q7 core groups.
    blk = np.concatenate(
        [idx[e].reshape(K // 16, 16).T for e in range(HEADS)], axis=1)
    gidx_sb = np.ascontiguousarray(np.tile(blk, (8, 1)), dtype=np.int16)
    return yt, gidx_sb


def kernel(Y, Ind, T, W):
    Y = np.asarray(Y, dtype=np.float32)
    Ind = np.asarray(Ind)
    W = np.asarray(W, dtype=np.float32)
    assert int(T) == T_SLOTS and Y.shape == (B, HEADS, K, HEAD_DIM)

    w_in = np.ascontiguousarray(
        W.transpose(1, 0, 2).reshape(HEAD_DIM, HEADS * OUT_DIM)
    ).astype(np.float16)

    in_maps = []
    for b in range(B):
        yt, gidx_sb = _prep_core_inputs(Y[b], Ind[b])
        in_maps.append({"yt": yt, "w": w_in, "gidx": gidx_sb})
    nc = _get_program()

    # The first execution of a freshly compiled NEFF occasionally wedges a
    # core; a retry on a fresh execute has been observed to recover.
    last_exc = None
    for attempt in range(3):
        try:
            res = run_bass_kernel_spmd(
                nc, in_maps, core_ids=list(range(NCORES)),
                trace=os.environ.get("ES_TRACE", "0") == "1",
            )
            break
        except Exception as exc:  # noqa: BLE001 - device flake, retry
            last_exc = exc
            import time as _time
            _time.sleep(2.0)
    else:
        raise last_exc
    kernel.last_results = res
    out = np.empty((B, T_SLOTS, OUT_DIM), dtype=np.float32)
    for b in range(B):
        for c in range(NCHAINS):
            acc = res.results[b][f"out{c}_0"].astype(np.float32)
            for p in range(1, NPARITY):
                acc += res.results[b][f"out{c}_{p}"].astype(np.float32)
            out[b, :, c * CW:(c + 1) * CW] = acc
    return out


if __name__ == "__main__":
    # quick self-check against a numpy reference
    rng = np.random.default_rng(0)
    Y = rng.standard_normal((B, HEADS, K, HEAD_DIM)).astype(np.float32)
    Ind = rng.integers(0, T_SLOTS, (B, HEADS, K)).astype(np.int32)
    bound = 1.0 / np.sqrt(OUT_DIM * HEADS)
    W = rng.uniform(-bound, bound, (HEADS, HEAD_DIM, OUT_DIM)).astype(np.float32)
    got = kernel(Y, Ind, T_SLOTS, W)
    X = np.einsum("bekj,eji->beki", Y.astype(np.float64), W.astype(np.float64))
    exp = np.zeros((B, T_SLOTS, OUT_DIM))
    for b in range(B):
        np.add.at(exp[b], Ind[b].reshape(-1), X[b].reshape(-1, OUT_DIM))
    err = np.linalg.norm(got - exp) / np.linalg.norm(exp)
    print(f"rel err {err:.3e}")
